# revision 11
# baseline (speedup 1.0000x reference)
"""Trainium2 Bass kernel for nn_Decoder: LSTM cell + causal attention + classifier.

Strategy (8 NeuronCores, data-parallel over batch, 4 rows/core):
  Phase A : feat_proj[t] = W_ih @ feat_t + (b_ih+b_hh), parallel matmuls,
            stored transposed-per-timestep in DRAM scratch.
  Scan    : sequential LSTM recurrence; W_hh stationary (bf16, fast weight
            load), h moving.  Gate layout gates.T = [4H on partition-tiles,
            batch free] so the recurrence needs no transposes.
            sigmoid(x) = (1+tanh(x/2))/2 keeps the whole kernel in the
            exp_and_others ACT table (tanh+exp, zero table switches).
  Attn    : per 128-step q-block, causal attention against SBUF-resident
            encodings (kept in both [h,j] and [j,h] layouts), then the
            classifier matmul.  The Tile scheduler interleaves this with the
            scan to fill PE gaps.

State scaling trick: store H = 2*h, C = 2*c; fold the 0.5 into host-side
pre-scaled W_hh and Wq so sigmoid-via-tanh costs no extra elementwise ops.
"""

import math
import os as _os

import numpy as np
import ml_dtypes

import concourse.bass as bass
import concourse.bacc as bacc
import concourse.mybir as mybir
import concourse.tile as tile
from concourse.bass import ts, ds
from concourse.masks import make_identity

BF16 = mybir.dt.bfloat16
F32 = mybir.dt.float32
AF = mybir.ActivationFunctionType
ALU = mybir.AluOpType

N_CORES = 8
BS = 4          # batch rows per core
H = 512
Fdim = 512
V = 1024
G4 = 4 * H      # 2048
KC = H // 128   # 4 contraction chunks
MT = G4 // 128  # 16 gate tiles
TB = 128        # timestep block size (q-block)
HB = 64         # scan half-block (fpb/hbt tile width)
UNROLL_SCAN = _os.environ.get("UNROLL_SCAN", "1") == "1"
SKIP_ATTN = _os.environ.get("SKIP_ATTN", "0") == "1"
SKIP_SCAN = _os.environ.get("SKIP_SCAN", "0") == "1"


def build(nc: bass.Bass, S: int):
    NB = S // TB
    assert S % TB == 0

    feats = nc.declare_dram_parameter("feats", [BS, S, Fdim], F32, isOutput=False)
    encs = nc.declare_dram_parameter("encs", [BS, S, H], F32, isOutput=False)
    w_ihT = nc.declare_dram_parameter("w_ihT", [Fdim, G4], BF16, isOutput=False)
    w_hhT = nc.declare_dram_parameter("w_hhT", [H, G4], BF16, isOutput=False)
    wqT = nc.declare_dram_parameter("wqT", [H, H], BF16, isOutput=False)
    wcT = nc.declare_dram_parameter("wcT", [H, V], BF16, isOutput=False)
    bias2 = nc.declare_dram_parameter("bias2", [MT, 128], F32, isOutput=False)
    bcv = nc.declare_dram_parameter("bcv", [1, V], BF16, isOutput=False)
    h0p = nc.declare_dram_parameter("h0p", [128, KC * BS], F32, isOutput=False)
    c0p = nc.declare_dram_parameter("c0p", [128, KC * BS], F32, isOutput=False)
    logits = nc.declare_dram_parameter("logits", [BS, S, V], F32, isOutput=True)

    fpT = nc.dram_tensor("fpT", [NB, MT, BS, 128, TB], BF16)

    sc_scale = 1.0 / math.sqrt(H)

    with tile.TileContext(nc) as tc:
        with (
            tc.tile_pool(name="const", bufs=1) as const,
            tc.tile_pool(name="enc_pool", bufs=1) as encp,
            tc.tile_pool(name="ld", bufs=3) as ld,
            tc.tile_pool(name="scan_state", bufs=1) as scanst,
            tc.tile_pool(name="hbt", bufs=4) as hbtp,
            tc.tile_pool(name="st", bufs=2) as st,
            tc.tile_pool(name="ps_tr", bufs=2, space="PSUM") as ps_tr,
            tc.tile_pool(name="ps_fi", bufs=1, space="PSUM") as ps_fi,
            tc.tile_pool(name="ps_gg", bufs=1, space="PSUM") as ps_gg,
            tc.tile_pool(name="ps_oo", bufs=1, space="PSUM") as ps_oo,
            tc.tile_pool(name="ps_sc", bufs=2, space="PSUM") as ps_sc,
        ):
            # ---------------- constants ----------------
            ident_b = const.tile([128, 128], BF16)
            make_identity(nc, ident_b)
            # causal keep-mask: cmask[p, x] = 1.0 if x <= p else 0.0
            cmask = const.tile([128, 128], BF16)
            nc.gpsimd.memset(cmask, 1.0)
            nc.gpsimd.affine_select(
                out=cmask, in_=cmask, compare_op=ALU.is_ge, fill=0.0,
                base=0, pattern=[[-1, 128]], channel_multiplier=1,
            )
            whh_sb = const.tile([128, KC, G4], BF16)
            nc.sync.dma_start(
                out=whh_sb, in_=w_hhT.rearrange("(kc p) m -> p kc m", p=128)
            )
            wq_sb = const.tile([128, KC, H], BF16)
            nc.sync.dma_start(out=wq_sb, in_=wqT.rearrange("(kc p) m -> p kc m", p=128))
            wc_sb = const.tile([128, KC, V], BF16)
            nc.sync.dma_start(out=wc_sb, in_=wcT.rearrange("(kc p) m -> p kc m", p=128))
            bias_sb = const.tile([128, MT], F32)
            nc.sync.dma_start(out=bias_sb, in_=bias2.rearrange("m p -> p m"))
            bc_sb = const.tile([128, V], BF16)
            nc.sync.dma_start(out=bc_sb, in_=bcv[:].to_broadcast([128, V]))

            encN = encp.tile([128, BS, NB, H], BF16)   # [j%128, b, jt, h]
            encT = encp.tile([128, BS, KC, S], BF16)   # [h%128, b, kc, j]

            # ------- scoped prep pool: enc relayout + phase A -------
            with tc.tile_pool(name="phA", bufs=1) as phA:
                ident_f = phA.tile([128, 128], F32)
                make_identity(nc, ident_f)
                wih_sb = phA.tile([128, KC, G4], BF16)
                nc.sync.dma_start(
                    out=wih_sb, in_=w_ihT.rearrange("(kc p) m -> p kc m", p=128)
                )
                with tc.tile_pool(name="phAw", bufs=3) as phAw:
                    for b in range(BS):
                        for jt in range(NB):
                            ef = phAw.tile([128, H], F32, tag="encld")
                            nc.sync.dma_start(out=ef, in_=encs[b, ts(jt, 128), :])
                            nc.vector.tensor_copy(encN[:, b, jt, :], ef)
                            for kc in range(KC):
                                pt = ps_tr.tile([128, 128], F32, tag="psx")
                                nc.tensor.transpose(pt, ef[:, ts(kc, 128)], ident_f)
                                nc.vector.tensor_copy(
                                    encT[:, b, kc, ts(jt, 128)], pt
                                )
                    for tb in range(NB):
                        for b in range(BS):
                            fnat = phAw.tile([128, Fdim], F32, tag="fnat")
                            nc.sync.dma_start(out=fnat, in_=feats[b, ts(tb, 128), :])
                            ftT = phAw.tile([128, KC, 128], BF16, tag="ftT")
                            for kc in range(KC):
                                pt = ps_tr.tile([128, 128], F32, tag="psx")
                                nc.tensor.transpose(pt, fnat[:, ts(kc, 128)], ident_f)
                                nc.vector.tensor_copy(ftT[:, kc, :], pt)
                            for m in range(MT):
                                pa = ps_tr.tile([128, 512], F32, tag="psx")
                                for kc in range(KC):
                                    nc.tensor.matmul(
                                        pa[:, 0:128],
                                        lhsT=wih_sb[:, kc, ts(m, 128)],
                                        rhs=ftT[:, kc, :],
                                        start=(kc == 0),
                                        stop=(kc == KC - 1),
                                    )
                                fo = phAw.tile([128, 128], BF16, tag="fo")
                                nc.scalar.activation(
                                    fo, pa[:, 0:128], AF.Identity,
                                    bias=bias_sb[:, m : m + 1],
                                )
                                nc.sync.dma_start(out=fpT[tb, m, b], in_=fo)

            # ---------------- scan + attention ----------------
            with (
                tc.tile_pool(name="fpbp", bufs=2) as fpbp,
                tc.tile_pool(name="att", bufs=2) as att,
                tc.tile_pool(name="att1", bufs=1) as att1,
            ):
                hinit = scanst.tile([128, KC, BS, 1], BF16)
                C = scanst.tile([128, KC, BS, 1], F32)
                htmp = ld.tile([128, KC * BS], F32, tag="h0ld")
                nc.sync.dma_start(out=htmp, in_=h0p[:])
                nc.vector.tensor_copy(
                    hinit, htmp.rearrange("p (k b one) -> p k b one", b=BS, one=1)
                )
                nc.sync.dma_start(
                    out=C, in_=c0p[:].rearrange("p (k b one) -> p k b one", b=BS, one=1)
                )

                def emit_step(hbt, fpb, idx):
                    """One LSTM step.  h history lives directly in hbt
                    (HB+1 slots; slot 0 is the carry-in): matmuls read slot
                    idx, the final stt writes slot idx+1 — no ping-pong.

                    Gate groups land in separate PSUM banks so the i,f
                    ADD+TANH overlaps the g/o matmul stream (bank-level
                    deps).  g and o accumulate fpb via an identity matmul
                    (static APs under UNROLL), so their TANH reads PSUM
                    directly.  Host pre-doubles the g-gate weights/bias so
                    every TANH uses scale 0.5."""
                    pfi = ps_fi.tile([128, 8, BS, 1], F32, tag="pfi")
                    pgg = ps_gg.tile([128, 4, BS, 1], F32, tag="pgg")
                    poo = ps_oo.tile([128, 4, BS, 1], F32, tag="poo")
                    for m in range(8):          # i (0:4), f (4:8)
                        for kc in range(KC):
                            nc.tensor.matmul(
                                pfi[:, m, :, :],
                                lhsT=whh_sb[:, kc, ts(m, 128)],
                                rhs=hbt[:, kc, :, ds(idx, 1)],
                                start=(kc == 0),
                                stop=(kc == KC - 1),
                            )
                    gfi = st.tile([128, 8, BS, 1], F32, tag="gfi")
                    Tfi = st.tile([128, 8, BS, 1], F32, tag="tfi")
                    nc.vector.tensor_add(gfi, pfi, fpb[:, 0:8, :, ds(idx, 1)])
                    nc.scalar.activation(Tfi, gfi, AF.Tanh, scale=0.5)
                    for m in range(8, 12):      # g
                        for kc in range(KC):
                            nc.tensor.matmul(
                                pgg[:, m - 8, :, :],
                                lhsT=whh_sb[:, kc, ts(m, 128)],
                                rhs=hbt[:, kc, :, ds(idx, 1)],
                                start=(kc == 0),
                                stop=False,
                            )
                        nc.tensor.matmul(
                            pgg[:, m - 8, :, :],
                            lhsT=ident_b,
                            rhs=fpb[:, m, :, ds(idx, 1)],
                            start=False,
                            stop=True,
                        )
                    Tg = st.tile([128, KC, BS, 1], F32, tag="tg")
                    nc.scalar.activation(Tg, pgg, AF.Tanh, scale=0.5)
                    for m in range(12, 16):     # o
                        for kc in range(KC):
                            nc.tensor.matmul(
                                poo[:, m - 12, :, :],
                                lhsT=whh_sb[:, kc, ts(m, 128)],
                                rhs=hbt[:, kc, :, ds(idx, 1)],
                                start=(kc == 0),
                                stop=False,
                            )
                        nc.tensor.matmul(
                            poo[:, m - 12, :, :],
                            lhsT=ident_b,
                            rhs=fpb[:, m, :, ds(idx, 1)],
                            start=False,
                            stop=True,
                        )
                    u = st.tile([128, KC, BS, 1], F32, tag="u")
                    v = st.tile([128, KC, BS, 1], F32, tag="v")
                    tcs = st.tile([128, KC, BS, 1], F32, tag="tcs")
                    To = st.tile([128, KC, BS, 1], F32, tag="to")
                    nc.vector.scalar_tensor_tensor(
                        out=u, in0=Tfi[:, 4:8, :, :], scalar=1.0, in1=C,
                        op0=ALU.add, op1=ALU.mult,
                    )
                    nc.vector.scalar_tensor_tensor(
                        out=v, in0=Tfi[:, 0:4, :, :], scalar=1.0, in1=Tg,
                        op0=ALU.add, op1=ALU.mult,
                    )
                    nc.vector.scalar_tensor_tensor(
                        out=C, in0=u, scalar=0.5, in1=v, op0=ALU.mult, op1=ALU.add
                    )
                    nc.scalar.activation(tcs, C, AF.Tanh, scale=0.5)
                    nc.scalar.activation(To, poo, AF.Tanh, scale=0.5)
                    nc.vector.scalar_tensor_tensor(
                        out=hbt[:, :, :, ds(idx + 1, 1)], in0=To, scalar=1.0,
                        in1=tcs, op0=ALU.add, op1=ALU.mult,
                    )

                prev_hbt = None
                for tb in range(NB):
                    halves = []
                    for half in range(TB // HB if not SKIP_SCAN else 0):
                        hbt = hbtp.tile([128, KC, BS, HB + 1], BF16, tag="hbt")
                        halves.append(hbt)
                        # carry-in: previous half's last h (or h0)
                        if prev_hbt is None:
                            nc.vector.tensor_copy(hbt[:, :, :, 0:1], hinit)
                        else:
                            nc.vector.tensor_copy(
                                hbt[:, :, :, 0:1], prev_hbt[:, :, :, HB : HB + 1]
                            )
                        prev_hbt = hbt
                        fpb = fpbp.tile([128, MT, BS, HB], BF16, tag="fpb")
                        nc.sync.dma_start(
                            out=fpb,
                            in_=fpT[tb, :, :, :, ds(half * HB, HB)].rearrange(
                                "m b p u -> p m b u"
                            ),
                        )
                        if UNROLL_SCAN:
                            for ti in range(HB):
                                emit_step(hbt, fpb, ti)
                        else:
                            with tc.For_i(
                                0, HB, 8, hint_engines=(mybir.EngineType.PE,)
                            ) as iv:
                                for q in range(8):
                                    emit_step(hbt, fpb, iv + q)

                    # ---- attention for this q-block ----
                    J = (tb + 1) * TB
                    for b in range(BS if not (SKIP_ATTN or SKIP_SCAN) else 0):
                        pq = ps_tr.tile([128, KC, 128], F32, tag="psx")
                        for mh in range(KC):
                            for hf in range(TB // HB):
                                for kc in range(KC):
                                    nc.tensor.matmul(
                                        pq[:, mh, ds(hf * HB, HB)],
                                        lhsT=wq_sb[:, kc, ts(mh, 128)],
                                        rhs=halves[hf][:, kc, b, ds(1, HB)],
                                        start=(kc == 0),
                                        stop=(kc == KC - 1),
                                    )
                        qT = att.tile([128, KC, 128], BF16, tag="qT")
                        nc.vector.tensor_copy(qT, pq)

                        e_sb = att1.tile([128, 2048], BF16, tag="esb")
                        den = st.tile([128, 1], F32, tag="den")
                        den2 = st.tile([128, 1], F32, tag="den2")
                        nprev = tb * TB
                        for c0 in range(0, nprev, 512):
                            c1 = min(nprev, c0 + 512)
                            pss = ps_sc.tile([128, 512], F32, tag="pscore")
                            for kc in range(KC):
                                nc.tensor.matmul(
                                    pss[:, 0 : c1 - c0],
                                    lhsT=qT[:, kc, :],
                                    rhs=encT[:, b, kc, c0:c1],
                                    start=(kc == 0),
                                    stop=(kc == KC - 1),
                                )
                            dpart = st.tile([128, 1], F32, tag="dpart")
                            nc.scalar.activation(
                                e_sb[:, c0:c1], pss[:, 0 : c1 - c0], AF.Exp,
                                scale=sc_scale, accum_out=dpart,
                            )
                            if c0 == 0:
                                nc.vector.tensor_copy(den, dpart)
                            else:
                                nc.vector.tensor_add(den, den, dpart)
                        pssd = ps_sc.tile([128, 512], F32, tag="pscore")
                        for kc in range(KC):
                            nc.tensor.matmul(
                                pssd[:, 0:TB],
                                lhsT=qT[:, kc, :],
                                rhs=encT[:, b, kc, ts(tb, TB)],
                                start=(kc == 0),
                                stop=(kc == KC - 1),
                            )
                        nc.scalar.activation(
                            e_sb[:, ts(tb, TB)], pssd[:, 0:TB], AF.Exp,
                            scale=sc_scale,
                        )
                        nc.vector.tensor_mul(
                            e_sb[:, ts(tb, TB)], e_sb[:, ts(tb, TB)], cmask
                        )
                        nc.vector.reduce_sum(
                            den2, e_sb[:, ts(tb, TB)], axis=mybir.AxisListType.X
                        )
                        if tb > 0:
                            nc.vector.tensor_add(den, den, den2)
                        else:
                            nc.vector.tensor_copy(den, den2)
                        recip = st.tile([128, 1], F32, tag="recip")
                        nc.vector.reciprocal(recip, den)
                        nc.vector.tensor_scalar_mul(e_sb[:, 0:J], e_sb[:, 0:J], recip)

                        pav = ps_tr.tile([128, H], F32, tag="psx")
                        for jt in range(tb + 1):
                            pt = ps_tr.tile([128, 128], BF16, tag="psx")
                            nc.tensor.transpose(pt, e_sb[:, ts(jt, 128)], ident_b)
                            aT = att.tile([128, 128], BF16, tag="attT")
                            nc.vector.tensor_copy(aT, pt)
                            nc.tensor.matmul(
                                pav,
                                lhsT=aT,
                                rhs=encN[:, b, jt, :],
                                start=(jt == 0),
                                stop=(jt == tb),
                            )
                        av_sb = att.tile([128, H], BF16, tag="avsb")
                        nc.vector.tensor_copy(av_sb, pav)
                        avT = att.tile([128, KC, 128], BF16, tag="avT")
                        for kc in range(KC):
                            pt = ps_tr.tile([128, 128], BF16, tag="psx")
                            nc.tensor.transpose(pt, av_sb[:, ts(kc, 128)], ident_b)
                            nc.vector.tensor_copy(avT[:, kc, :], pt)
                        lg = att1.tile([128, V], F32, tag="lg")
                        for nv in range(V // 512):
                            psl = ps_tr.tile([128, 512], F32, tag="psx")
                            for kc in range(KC):
                                nc.tensor.matmul(
                                    psl,
                                    lhsT=avT[:, kc, :],
                                    rhs=wc_sb[:, kc, ts(nv, 512)],
                                    start=(kc == 0),
                                    stop=(kc == KC - 1),
                                )
                            nc.vector.tensor_add(
                                lg[:, ts(nv, 512)], psl, bc_sb[:, ts(nv, 512)]
                            )
                        nc.sync.dma_start(out=logits[b, ts(tb, 128), :], in_=lg)
    return nc


def make_nc(S):
    nc = bacc.Bacc("TRN2", target_bir_lowering=False)
    build(nc, S)
    nc.compile()
    return nc


def _host_inputs(inputs, core):
    bf = ml_dtypes.bfloat16
    W_ih = inputs["W_ih"].astype(np.float32)
    W_hh = inputs["W_hh"].astype(np.float32)
    Wq = inputs["Wq"].astype(np.float32)
    Wc = inputs["Wc"].astype(np.float32)
    b_ih = inputs["b_ih"].astype(np.float32)
    b_hh = inputs["b_hh"].astype(np.float32)
    bc = inputs["bc"].astype(np.float32)
    h0 = inputs["h0"].astype(np.float32)
    c0 = inputs["c0"].astype(np.float32)
    sl = slice(core * BS, (core + 1) * BS)

    def pack_state(x, scale):
        # [BS, H] -> [128, KC*BS], col 4k+b = x[b, 128k+p]*scale
        xs = (x[sl] * scale).T.reshape(KC, 128, BS)  # [k, p, b]
        return np.ascontiguousarray(np.transpose(xs, (1, 0, 2)).reshape(128, KC * BS))

    # Per-gate-row scale: 0.5 compensates the H=2h state doubling; the g
    # block (rows 2H:3H) gets an extra 2x so the kernel can use a single
    # tanh(0.5*x) for every gate (tanh(0.5*(2g)) == tanh(g)).
    gsc = np.ones((4 * H, 1), np.float32)
    gsc[2 * H : 3 * H] = 2.0
    return {
        "feats": np.ascontiguousarray(inputs["features"][sl]).astype(np.float32),
        "encs": np.ascontiguousarray(inputs["encodings"][sl]).astype(np.float32),
        "w_ihT": np.ascontiguousarray((gsc * W_ih).T).astype(bf),
        "w_hhT": np.ascontiguousarray((gsc * 0.5 * W_hh).T).astype(bf),
        "wqT": np.ascontiguousarray((0.5 * Wq).T).astype(bf),
        "wcT": np.ascontiguousarray(Wc.T).astype(bf),
        "bias2": np.ascontiguousarray((gsc[:, 0] * (b_ih + b_hh)).reshape(MT, 128)),
        "bcv": np.ascontiguousarray(bc.reshape(1, V)).astype(bf),
        "h0p": pack_state(h0, 2.0),
        "c0p": pack_state(c0, 2.0),
    }


def kernel(**inputs) -> np.ndarray:
    from concourse.bass_utils import run_bass_kernel_spmd

    S = inputs["features"].shape[1]
    nc = make_nc(S)
    in_maps = [_host_inputs(inputs, j) for j in range(N_CORES)]
    res = run_bass_kernel_spmd(nc, in_maps, list(range(N_CORES))).results
    out = np.concatenate([r["logits"] for r in res], axis=0)
    return out.astype(np.float32)

